# revision 68
# baseline (speedup 1.0000x reference)
"""MultiHeadCrossAttention kernel for 8 Trainium2 NeuronCores.

Sharding: pure data-parallel over batch (B=8 -> 1 batch element per core).

Per-core design (v2):
  - Activations are transposed and cast to bf16 on the HOST (xT/kT/vT are
    feature-major [E, L]); no on-chip transposes for the projections.
  - All matmul operands bf16 (weights host-cast); PSUM accumulation fp32.
  - qT/kT feature-major [E, L]; v token-major with a ones-column per head
    (softmax denominator rides the attn@v matmul).
  - q-blocks of 128 rows; per head: 8 score matmuls -> one [128,1024] exp on
    ScalarE (bf16 out) -> 8 av matmuls; probs normalized on DVE in 2x bf16
    mode; attn_weights mean accumulated via transpose-accumulate matmuls
    directly into natural [q,k] PSUM.
  - inv/gamma/beta broadcasts on GpSimd (partition_broadcast); gamma/beta
    applies on GpSimd; LN rstd = exp(-0.5*ln(var+eps)) so ScalarE only ever
    uses {Exp, Ln, Copy} (single activation table).
  - Head loop software-pipelined (scores s / av s-1 / W-acc s-5) so the PE
    stream never stalls on ScalarE.
"""

import numpy as np
from contextlib import ExitStack

import concourse.bacc as bacc
import concourse.bass as bass
import concourse.tile as tile
from concourse import mybir
from concourse.bass_utils import run_bass_kernel_spmd
from concourse.masks import make_identity

_orig_get_act_tables = bacc.get_activation_tables
_PREFERRED_TABLE = "natural_log_exp_and_others"


def _patched_get_act_tables(arch):
    """Steer the act-table chooser so Exp and Ln both resolve to the one
    table containing both, avoiding per-q-block table reloads. Only the
    chooser sees the stripped sets; act_func_set_id indices (and the real
    tables loaded at runtime) are unchanged."""
    tabs = dict(_orig_get_act_tables(arch))
    if _PREFERRED_TABLE not in tabs:
        return tabs
    exp_f = mybir.ActivationFunctionType.Exp
    ln_f = mybir.ActivationFunctionType.Ln
    out = {}
    for name, funcs in tabs.items():
        if name != _PREFERRED_TABLE:
            funcs = funcs - {exp_f, ln_f}
        out[name] = funcs
    return out


bacc.get_activation_tables = _patched_get_act_tables

E = 1024
H = 16
DH = 64
L = 1024
P = 128
QB = 128          # q-block size
NQB = L // QB     # 8
NKT = L // P      # 8 k-tiles
NEC = E // P      # 8 feature chunks
VS = H * (DH + 1)  # 1040 v columns per k-chunk (65 per head)
LN_EPS = 1e-5

F32 = mybir.dt.float32
BF16 = mybir.dt.bfloat16
AF = mybir.ActivationFunctionType
OP = mybir.AluOpType


def _emit(nc, tc, io):
    xT, kTin, vTin = io["xT"], io["kTin"], io["vTin"]
    xnat = io["xnat"]
    wqT_d, wkT_d, wvT_d, woT_d = io["wqT"], io["wkT"], io["wvT"], io["woT"]
    bqk_d, brow_d = io["bqk"], io["brow"]
    y_out, w_out = io["y_out"], io["w_out"]

    ctx = tc.ctx
    ctx.enter_context(nc.allow_low_precision("bf16 attention"))

    const = ctx.enter_context(tc.tile_pool(name="const", bufs=1))
    persist = ctx.enter_context(tc.tile_pool(name="persist", bufs=1))

    ident_f = const.tile([P, P], F32)
    make_identity(nc, ident_f[:])
    ident = const.tile([P, P], BF16)
    nc.vector.tensor_copy(ident[:], ident_f[:])
    identH = const.tile([P, P], BF16)  # I/H for the attn-weights mean
    nc.scalar.mul(identH[:], ident_f[:], 1.0 / H)
    ones1 = const.tile([1, P], BF16)
    nc.vector.memset(ones1[:], 1.0)
    eps_sb = const.tile([P, 1], F32)
    nc.vector.memset(eps_sb[:], LN_EPS)
    # hoist the single activation-table load to t=0 (ScalarE idle)
    scratch1 = const.tile([1, 1], F32)
    nc.scalar.activation(scratch1[:], eps_sb[0:1, 0:1], AF.Exp)

    # bqk: [128, 16] f32 (cols 0-7 = bq chunk m, cols 8-15 = bk chunk m)
    bqk = const.tile([P, 2 * NEC], F32)
    # brow: [1, 4096] bf16 = bv | bo | gamma | beta
    brow = const.tile([1, 4 * E], BF16)

    qT = persist.tile([P, NEC * L], BF16)
    kT = persist.tile([P, NEC * L], BF16)
    v_sb = persist.tile([P, NKT * VS], BF16)
    woT = persist.tile([P, NEC * E], BF16)
    gamma_bc = persist.tile([P, E], BF16)
    beta_bc = persist.tile([P, E], BF16)

    # ones columns of v_sb (denominator trick): one strided memset
    nc.vector.memset(
        v_sb[:].rearrange("p (n d) -> p n d", d=DH + 1)[:, :, DH:DH + 1], 1.0)

    # ---------------- phase 1: projections ----------------
    with tc.tile_pool(name="ld", bufs=2) as ld_pool, \
         tc.tile_pool(name="wt", bufs=2) as wt_pool, \
         tc.tile_pool(name="pp", bufs=4, space="PSUM") as pp_pool:

        # order: k first, then v, then q (phase 2 needs kT/v first; q-proj
        # n=0 halves emitted before n=1 so early q-blocks can start).
        for ti, (src, w_d) in enumerate([(kTin, wkT_d), (vTin, wvT_d), (xT, wqT_d)]):
            aT = ld_pool.tile([P, NEC * L], BF16, tag="ld", name=f"aT_{ti}")
            wt = wt_pool.tile([P, NEC * E], BF16, tag="wt", name=f"wt_{ti}")
            if ti == 0:
                # quarter DMAs (chunk-halves x column-halves) ordered so the
                # first matmuls start after two 0.5MB transfers and the PE
                # never outruns the stream
                def wq(cs, hs):
                    return (wt[:].rearrange("p (c e) -> p c e", e=E)[:, cs, hs:hs + 512],
                            w_d.rearrange("(c p) e -> p c e", p=P)[:, cs, hs:hs + 512])
                def aq(cs, hs):
                    return (aT[:].rearrange("p (c l) -> p c l", l=L)[:, cs, hs:hs + 512],
                            src.rearrange("(c p) l -> p c l", p=P)[:, cs, hs:hs + 512])
                c01, c23, c03 = slice(0, 2), slice(2, 4), slice(0, 4)
                c45, c67, c47 = slice(4, 6), slice(6, 8), slice(4, 8)
                order = [wq(c01, 0), aq(c01, 0), wq(c23, 0), aq(c23, 0),
                         wq(c45, 0), aq(c45, 0), wq(c67, 0), aq(c67, 0),
                         aq(c03, 512), aq(c47, 512), wq(c03, 512), wq(c47, 512)]
                for o, i in order:
                    nc.sync.dma_start(out=o, in_=i)
                nc.sync.dma_start(out=bqk[:], in_=bqk_d[:, :])
                for r in range(4):
                    nc.sync.dma_start(out=brow[:, E * r:E * (r + 1)],
                                      in_=brow_d[r:r + 1, :])
            else:
                nc.sync.dma_start(
                    out=aT[:].rearrange("p (c l) -> p c l", l=L),
                    in_=src.rearrange("(c p) l -> p c l", p=P))
                nc.sync.dma_start(
                    out=wt[:].rearrange("p (c e) -> p c e", e=E),
                    in_=w_d.rearrange("(c p) e -> p c e", p=P))
            if ti == 0:
                # match the wtL/aTL/aTR/wtR DMA order above
                tiles16 = ([(m, 0) for m in range(4)] + [(m, 1) for m in range(4)]
                           + [(m, 0) for m in range(4, NEC)]
                           + [(m, 1) for m in range(4, NEC)])
            else:
                tiles16 = [(m, n) for n in range(2) for m in range(NEC)]
            for g in range(0, 16, 4):
                grp = tiles16[g:g + 4]
                psums = [
                    pp_pool.tile([P, 512], F32, tag="pp", name=f"pp_{ti}_{g}_{i}")
                    for i in range(len(grp))
                ]
                for c in range(NEC):
                    for i, (m, n) in enumerate(grp):
                        if ti != 1:  # q/k: feature-major out [e', l]
                            lhsT = wt[:, E * c + P * m: E * c + P * (m + 1)]
                            rhs = aT[:, L * c + 512 * n: L * c + 512 * (n + 1)]
                        else:        # v: token-major out [l, e']
                            lhsT = aT[:, L * c + P * m: L * c + P * (m + 1)]
                            rhs = wt[:, E * c + 512 * n: E * c + 512 * (n + 1)]
                        nc.tensor.matmul(
                            psums[i][:], lhsT, rhs,
                            start=(c == 0), stop=(c == NEC - 1),
                        )
                evict_order = list(enumerate(grp))
                if ti == 2 and g == 12:
                    # last group: evict the tiles whose psum banks the
                    # first phase-2 scores tile reuses first
                    evict_order = evict_order[::-1]
                for i, (m, n) in evict_order:
                    if ti != 1:
                        # bqk cols: 0-7 = bq, 8-15 = bk
                        dst_t = kT if ti == 0 else qT
                        bcol = NEC + m if ti == 0 else m
                        dst = dst_t[:, L * m + 512 * n: L * m + 512 * (n + 1)]
                        if ti == 2 and n == 1 and m % 2 == 0:
                            # split late q-proj evicts DVE/ACT so neither
                            # engine delays the first q-blocks
                            nc.vector.tensor_scalar_add(
                                out=dst, in0=psums[i][:],
                                scalar1=bqk[:, bcol:bcol + 1],
                            )
                        else:
                            nc.scalar.activation(
                                dst, psums[i][:], AF.Identity,
                                bias=bqk[:, bcol:bcol + 1],
                            )
                    else:
                        # v bias folded into the residual host-side
                        # (sum(probs)=1 makes it a constant shift of attn)
                        dst = v_sb[:, VS * m + 520 * n: VS * m + 520 * (n + 1)]
                        nc.vector.tensor_copy(
                            out=dst.rearrange("p (h d) -> p h d", d=DH + 1)[:, :, 0:DH],
                            in_=psums[i][:].rearrange("p (h d) -> p h d", d=DH),
                        )

        # out-proj weights (loaded during phase 1 tail)
        for c in range(NEC):
            nc.sync.dma_start(out=woT[:, E * c:E * (c + 1)],
                              in_=woT_d[P * c:P * (c + 1), :])
        # gamma/beta broadcast on gpsimd
        nc.gpsimd.partition_broadcast(gamma_bc[:], brow[0:1, 2 * E:3 * E])
        nc.gpsimd.partition_broadcast(beta_bc[:], brow[0:1, 3 * E:4 * E])

    # ---------------- phase 2: attention + out_proj + LN ----------------
    with tc.tile_pool(name="wnp", bufs=1, space="PSUM") as wn_pool, \
         tc.tile_pool(name="avp", bufs=1, space="PSUM") as av_pool, \
         tc.tile_pool(name="scp", bufs=3, space="PSUM") as sc_pool, \
         tc.tile_pool(name="expp", bufs=8) as exp_pool, \
         tc.tile_pool(name="prp", bufs=18) as probs_pool, \
         tc.tile_pool(name="atp", bufs=2) as attnT_pool, \
         tc.tile_pool(name="avsp", bufs=2) as avs_pool, \
         tc.tile_pool(name="ibp", bufs=8) as invbc_pool, \
         tc.tile_pool(name="ivp", bufs=2) as inv_pool, \
         tc.tile_pool(name="xqp", bufs=2) as xq_pool, \
         tc.tile_pool(name="yp", bufs=2) as y_pool, \
         tc.tile_pool(name="acq", bufs=2) as accq_pool, \
         tc.tile_pool(name="wnat", bufs=2) as wnat_pool, \
         tc.tile_pool(name="small", bufs=2) as small:

        SKEW_AV = 4    # av(h) emitted at slot h+3 (covers ScalarE backlog
        # around group ends, where the av eviction queues behind the exps)
        SKEW_W = 8     # W-acc pass A (h) at slot h+8 (after normalize)

        def emit_scores(qb, state, h):
            q0 = QB * qb
            hb, hc = (h % 2) * DH, h // 2
            sc = sc_pool.tile([P, L], F32, tag="sc", name=f"sc_{qb}_{h}")
            state["scs"].append(sc)
            for kt in range(NKT):
                nc.tensor.matmul(
                    sc[:, P * kt:P * (kt + 1)],
                    kT[hb:hb + DH, L * hc + P * kt: L * hc + P * (kt + 1)],
                    qT[hb:hb + DH, L * hc + q0: L * hc + q0 + QB],
                    start=True, stop=True,
                )
            expT = exp_pool.tile([P, L], BF16, tag="expT", name=f"expT_{qb}_{h}")
            state["exps"].append(expT)
            nc.scalar.activation(expT[:], sc[:], AF.Exp, scale=0.125)

        def emit_av(qb, state, h):
            exps, av4s, avss, inv4s, invbcs, probs = (
                state["exps"], state["av4s"], state["avss"],
                state["inv4s"], state["invbcs"], state["probs"],
            )
            g, hi = h // 4, h % 4
            if hi == 0:
                av4 = av_pool.tile([DH + 1, 4 * QB], F32, tag="av",
                                   name=f"av_{qb}_{g}")
                av4s.append(av4)
                inv4 = inv_pool.tile([1, 4 * QB], BF16, tag="inv",
                                     name=f"inv_{qb}_{g}")
                inv4s.append(inv4)
                avs = avs_pool.tile([DH, 4 * QB], BF16, tag="avs",
                                    name=f"avs_{qb}_{g}")
                avss.append(avs)
            av4, inv4, avs = av4s[g], inv4s[g], avss[g]
            expT = exps[h]
            for kt in range(NKT):
                nc.tensor.matmul(
                    av4[:, QB * hi:QB * (hi + 1)],
                    v_sb[:, VS * kt + (DH + 1) * h: VS * kt + (DH + 1) * (h + 1)],
                    expT[:, QB * kt:QB * (kt + 1)],
                    start=(kt == 0), stop=(kt == NKT - 1),
                )
            if hi == 3:
                # group complete: reciprocals, evict av to SBUF (frees the
                # single psum slot), broadcasts, normalize, attnT
                nc.vector.reciprocal(inv4[:], av4[DH:DH + 1, :])
                nc.scalar.copy(avs[:], av4[0:DH, :])
                for hh in range(4 * g, 4 * g + 4):
                    hhi = hh % 4
                    ib = invbc_pool.tile([P, QB], BF16, tag="ib",
                                         name=f"ib_{qb}_{hh}")
                    invbcs.append(ib)
                    nc.gpsimd.partition_broadcast(
                        ib[:], inv4[0:1, QB * hhi:QB * (hhi + 1)])
                for hh in range(4 * g, 4 * g + 4):
                    hhb, hhc = (hh % 2) * DH, hh // 2
                    hhi = hh % 4
                    ib = invbcs[hh]
                    pr = probs_pool.tile([P, L], BF16, tag="pr",
                                         name=f"pr_{qb}_{hh}")
                    probs.append(pr)
                    iap = ib[:]
                    bc_ap = bass.AP(
                        tensor=iap.tensor, offset=iap.offset,
                        ap=[iap.ap[0], [0, NKT], iap.ap[1]],
                    )
                    nc.vector.tensor_tensor(
                        out=pr[:].rearrange("p (n d) -> p n d", d=QB),
                        in0=exps[hh][:].rearrange("p (n d) -> p n d", d=QB),
                        in1=bc_ap, op=OP.mult,
                    )
                    nc.gpsimd.tensor_tensor(
                        out=state["attnT"][hhb:hhb + DH, QB * hhc:QB * (hhc + 1)],
                        in0=avs[:, QB * hhi:QB * (hhi + 1)],
                        in1=ib[0:DH, :], op=OP.mult,
                    )

        NH = NQB * H  # 128 global head indices; one flat pipeline, no
        # per-q-block drain: the next block's scores start while the
        # previous block's softmax/W tail is still in flight.

        states = {}

        def get_state(qb):
            if qb not in states:
                q0 = QB * qb
                x_qb = xq_pool.tile([P, E], F32, tag="xq", name=f"xq_{qb}")
                nc.sync.dma_start(out=x_qb[:], in_=xnat[q0:q0 + QB, :])
                attnT = attnT_pool.tile([P, NEC * QB], BF16, tag="attnT",
                                        name=f"attnT_{qb}")
                states[qb] = dict(scs=[], exps=[], av4s=[], avss=[],
                                  inv4s=[], invbcs=[], probs=[], attnT=attnT,
                                  x_qb=x_qb, wnA=None)
            return states[qb]

        def extras_po_y(qb_e):
            st = states.get(qb_e)
            # out_proj burst + residual add
            po = sc_pool.tile([P, L], F32, tag="sc", name=f"po_{qb_e}")
            attnT = st["attnT"]
            for eb in range(2):
                for c in range(NEC):
                    nc.tensor.matmul(
                        po[:, 512 * eb:512 * (eb + 1)],
                        attnT[:, QB * c:QB * (c + 1)],
                        woT[:, E * c + 512 * eb: E * c + 512 * (eb + 1)],
                        start=(c == 0), stop=(c == NEC - 1),
                    )
            y_sb = y_pool.tile([P, E], F32, tag="y", name=f"y_{qb_e}")
            st["y_sb"] = y_sb
            for eb in range(2):
                nc.vector.tensor_tensor(
                    out=y_sb[:, 512 * eb:512 * (eb + 1)],
                    in0=po[:, 512 * eb:512 * (eb + 1)],
                    in1=st["x_qb"][:, 512 * eb:512 * (eb + 1)],
                    op=OP.add,
                )

        def extras_w(qb_e):
            # W: pass A eviction, pass B burst + eviction
            st = states.get(qb_e)
            accq = accq_pool.tile([P, L], BF16, tag="accq",
                                  name=f"accq_{qb_e}")
            st["accq"] = accq
            nc.scalar.copy(accq[:, 0:512], st["wnA"][:])
            if qb_e == NQB - 1:
                # last block: store the finished half immediately
                nc.scalar.dma_start(
                    out=w_out[QB * qb_e:QB * (qb_e + 1), 0:512],
                    in_=accq[:, 0:512])
            wnB = wn_pool.tile([P, 512], F32, tag="wn", name=f"wnB_{qb_e}")
            for h in range(H):
                nc.tensor.matmul(
                    wnB[:], identH[:], st["probs"][h][:, 512:1024],
                    start=(h == 0), stop=(h == H - 1),
                )
            nc.scalar.copy(accq[:, 512:1024], wnB[:])

        def extras_wdma(qb_e):
            # store W in k-major block layout; host reassembles. The last
            # block's store goes through the ScalarE queue so it is not
            # head-of-line blocked behind the final y stores.
            st = states.get(qb_e)
            if qb_e == NQB - 1:
                nc.scalar.dma_start(
                    out=w_out[QB * qb_e:QB * (qb_e + 1), 512:1024],
                    in_=st["accq"][:, 512:1024])
            else:
                nc.sync.dma_start(out=w_out[QB * qb_e:QB * (qb_e + 1), :],
                                  in_=st["accq"][:])
            del states[qb_e]

        def extras_ytail(qb_e):
            st = states.get(qb_e)
            q0e = QB * qb_e
            if True:
                # y tail: stats, LN, gamma/beta halves (DVE || Pool)
                y_sb = st["y_sb"]
                stats = small.tile([P, 2, 6], F32, tag="stats",
                                   name=f"st_{qb_e}")
                yg = y_sb[:].rearrange("p (s f) -> p s f", f=512)
                for sg in range(2):
                    nc.vector.bn_stats(out=stats[:, sg, :], in_=yg[:, sg, :])
                mv = small.tile([P, 2], F32, tag="mv", name=f"mv_{qb_e}")
                nc.vector.bn_aggr(out=mv[:], in_=stats[:])
                lnv = small.tile([P, 1], F32, tag="lnv", name=f"lnv_{qb_e}")
                nc.scalar.activation(lnv[:], mv[:, 1:2], AF.Ln, bias=eps_sb[:])
                rstd = small.tile([P, 1], F32, tag="rstd", name=f"rstd_{qb_e}")
                nc.scalar.activation(rstd[:], lnv[:], AF.Exp, scale=-0.5)
                y_bf = y_pool.tile([P, E], BF16, tag="ybf", name=f"ybf_{qb_e}")
                # gamma/beta split DVE || Pool; the last block gives Pool a
                # smaller share (nothing left to hide its latency behind)
                cut = 768 if qb_e == NQB - 1 else 512
                for sl, eng in ((slice(0, cut), nc.vector),
                                (slice(cut, E), nc.gpsimd)):
                    nc.vector.tensor_scalar(
                        out=y_sb[:, sl], in0=y_sb[:, sl],
                        scalar1=mv[:, 0:1], scalar2=rstd[:],
                        op0=OP.subtract, op1=OP.mult,
                    )
                    eng.tensor_tensor(
                        out=y_sb[:, sl], in0=y_sb[:, sl],
                        in1=gamma_bc[:, sl], op=OP.mult)
                    eng.tensor_tensor(
                        out=y_bf[:, sl], in0=y_sb[:, sl],
                        in1=beta_bc[:, sl], op=OP.add)
                    nc.sync.dma_start(out=y_out[q0e:q0e + QB, sl],
                                      in_=y_bf[:, sl])

        # steady-state extras fire at js = 16*qb + 18 + ph with the PE-heavy
        # work (out_proj + W passes) at ph 6; the last block has no
        # successor to overlap with, so its y path fires as soon as its
        # attnT completes
        for js in range(NH + 2 * H):
            off = js - H - 2  # qb whose window-end extras fire at this js
            qb_e, ph = divmod(off, H)
            if 0 <= qb_e < NQB - 1:
                if ph == 6:
                    extras_po_y(qb_e)
                    extras_w(qb_e)
                elif ph == 8:
                    extras_ytail(qb_e)
                elif ph == 9:
                    extras_wdma(qb_e)
            elif qb_e == NQB - 1:
                if ph == 0:
                    extras_po_y(qb_e)
                elif ph == 1:
                    extras_ytail(qb_e)
                elif ph == 4:
                    extras_w(qb_e)
                elif ph == 5:
                    extras_wdma(qb_e)

            ja = js - SKEW_AV
            group_end = 0 <= ja < NH and ja % 4 == 3
            if group_end:
                emit_av(ja // H, get_state(ja // H), ja % H)
            if js < NH:
                emit_scores(js // H, get_state(js // H), js % H)
            if not group_end and 0 <= ja < NH:
                emit_av(ja // H, get_state(ja // H), ja % H)
            # W pass A (kt 0-3); heads 0-1 deferred to the h2 slot so the
            # first wnA write follows the previous block's passB on PE
            jw = js - SKEW_W
            if 0 <= jw < NH:
                qb_w, h_w = divmod(jw, H)
                heads = [] if h_w < 3 else ([0, 1, 2, 3] if h_w == 3 else [h_w])
                if qb_w == NQB - 1:
                    # last block: compress the W-acc tail (its pass B has
                    # nothing to overlap with)
                    if h_w == 13:
                        heads = [13, 14, 15]
                    elif h_w > 13:
                        heads = []
                st = get_state(qb_w) if heads else None
                for hw in heads:
                    if hw == 0:
                        st["wnA"] = wn_pool.tile([P, 512], F32, tag="wn",
                                                 name=f"wnA_{qb_w}")
                    nc.tensor.matmul(
                        st["wnA"][:], identH[:], st["probs"][hw][:, 0:512],
                        start=(hw == 0), stop=(hw == H - 1),
                    )


_CACHED = None


def _build():
    global _CACHED
    if _CACHED is not None:
        return _CACHED
    nc = bacc.Bacc("TRN2", target_bir_lowering=False, debug=False, num_devices=8)
    io = {}
    for name in ["xT", "kTin", "vTin", "wqT", "wkT", "wvT", "woT"]:
        io[name] = nc.dram_tensor(name, [1024, 1024], BF16, kind="ExternalInput").ap()
    io["xnat"] = nc.dram_tensor("xnat", [1024, 1024], F32, kind="ExternalInput").ap()
    io["bqk"] = nc.dram_tensor("bqk", [128, 16], F32, kind="ExternalInput").ap()
    io["brow"] = nc.dram_tensor("brow", [4, 1024], BF16, kind="ExternalInput").ap()
    io["y_out"] = nc.dram_tensor("y_out", [1024, 1024], BF16, kind="ExternalOutput").ap()
    io["w_out"] = nc.dram_tensor("w_out", [1024, 1024], BF16, kind="ExternalOutput").ap()
    with tile.TileContext(nc) as tc:
        with ExitStack() as ctx:
            tc.ctx = ctx
            _emit(nc, tc, io)
    nc.compile()
    _CACHED = nc
    return nc


def kernel(query, key_t, value, in_proj_w, in_proj_b, out_proj_w, out_proj_b,
           ln_gamma, ln_beta, _trace=False, _tmpdir=None):
    import ml_dtypes
    bf16 = ml_dtypes.bfloat16

    query = np.ascontiguousarray(np.asarray(query, dtype=np.float32))
    key_t = np.asarray(key_t, dtype=np.float32)
    value = np.asarray(value, dtype=np.float32)
    # residual carries the out_proj bias AND the value bias's constant
    # contribution (sum(probs) = 1 -> attn = probs@v_core + bv, and
    # (bv @ wo^T) is a constant row): y = (query + bo + wo@bv) + attn@woT
    out_proj_w = np.asarray(out_proj_w, dtype=np.float32)
    bv_host = np.asarray(in_proj_b, np.float32)[2 * E:3 * E]
    xres = np.ascontiguousarray(
        query + (np.asarray(out_proj_b, np.float32)
                 + out_proj_w @ bv_host)[None, None, :])
    xT = np.ascontiguousarray(np.swapaxes(query, 1, 2)).astype(bf16)
    kT = np.ascontiguousarray(np.swapaxes(key_t, 1, 2)).astype(bf16)
    vT = np.ascontiguousarray(np.swapaxes(value, 1, 2)).astype(bf16)

    in_proj_w = np.asarray(in_proj_w, dtype=np.float32)
    wqT = np.ascontiguousarray(in_proj_w[0:E].T).astype(bf16)
    wkT = np.ascontiguousarray(in_proj_w[E:2 * E].T).astype(bf16)
    wvT = np.ascontiguousarray(in_proj_w[2 * E:3 * E].T).astype(bf16)
    woT = np.ascontiguousarray(np.asarray(out_proj_w, dtype=np.float32).T).astype(bf16)

    b = np.asarray(in_proj_b, dtype=np.float32)
    bq, bk, bv = b[0:E], b[E:2 * E], b[2 * E:3 * E]
    bqk = np.ascontiguousarray(
        np.concatenate([bq.reshape(NEC, P).T, bk.reshape(NEC, P).T], axis=1)
    ).astype(np.float32)  # [128, 16]
    brow = np.ascontiguousarray(np.stack([
        bv, np.asarray(out_proj_b, np.float32),
        np.asarray(ln_gamma, np.float32), np.asarray(ln_beta, np.float32),
    ])).astype(bf16)  # [4, 1024]

    nc = _build()
    in_maps = [
        dict(xT=xT[c], kTin=kT[c], vTin=vT[c], xnat=xres[c],
             wqT=wqT, wkT=wkT, wvT=wvT, woT=woT, bqk=bqk, brow=brow)
        for c in range(8)
    ]
    res = run_bass_kernel_spmd(
        nc, in_maps, core_ids=list(range(8)), trace=_trace, tmpdir=_tmpdir
    )
    y = np.stack([r["y_out"] for r in res.results]).astype(np.float32)
    # w_out rows hold k-major blocks: w_raw[qb*128+p, kt*128+qq] =
    # W[qb*128+qq, kt*128+p]
    w_raw = np.stack([r["w_out"] for r in res.results]).astype(np.float32)
    w = np.ascontiguousarray(
        w_raw.reshape(8, NQB, P, NKT, P).transpose(0, 1, 4, 3, 2)
        .reshape(8, L, L))
    kernel._last_result = res
    return y, w


# revision 70
# speedup vs baseline: 1.0177x; 1.0177x over previous
"""MultiHeadCrossAttention kernel for 8 Trainium2 NeuronCores.

Sharding: pure data-parallel over batch (B=8 -> 1 batch element per core).

Per-core design (v2):
  - Activations are transposed and cast to bf16 on the HOST (xT/kT/vT are
    feature-major [E, L]); no on-chip transposes for the projections.
  - All matmul operands bf16 (weights host-cast); PSUM accumulation fp32.
  - qT/kT feature-major [E, L]; v token-major with a ones-column per head
    (softmax denominator rides the attn@v matmul).
  - q-blocks of 128 rows; per head: 8 score matmuls -> one [128,1024] exp on
    ScalarE (bf16 out) -> 8 av matmuls; probs normalized on DVE in 2x bf16
    mode; attn_weights mean accumulated via transpose-accumulate matmuls
    directly into natural [q,k] PSUM.
  - inv/gamma/beta broadcasts on GpSimd (partition_broadcast); gamma/beta
    applies on GpSimd; LN rstd = exp(-0.5*ln(var+eps)) so ScalarE only ever
    uses {Exp, Ln, Copy} (single activation table).
  - Head loop software-pipelined (scores s / av s-1 / W-acc s-5) so the PE
    stream never stalls on ScalarE.
"""

import numpy as np
from contextlib import ExitStack

import concourse.bacc as bacc
import concourse.bass as bass
import concourse.tile as tile
from concourse import mybir
from concourse.bass_utils import run_bass_kernel_spmd
from concourse.masks import make_identity

_orig_get_act_tables = bacc.get_activation_tables
_PREFERRED_TABLE = "natural_log_exp_and_others"


def _patched_get_act_tables(arch):
    """Steer the act-table chooser so Exp and Ln both resolve to the one
    table containing both, avoiding per-q-block table reloads. Only the
    chooser sees the stripped sets; act_func_set_id indices (and the real
    tables loaded at runtime) are unchanged."""
    tabs = dict(_orig_get_act_tables(arch))
    if _PREFERRED_TABLE not in tabs:
        return tabs
    exp_f = mybir.ActivationFunctionType.Exp
    ln_f = mybir.ActivationFunctionType.Ln
    out = {}
    for name, funcs in tabs.items():
        if name != _PREFERRED_TABLE:
            funcs = funcs - {exp_f, ln_f}
        out[name] = funcs
    return out


bacc.get_activation_tables = _patched_get_act_tables

E = 1024
H = 16
DH = 64
L = 1024
P = 128
QB = 128          # q-block size
NQB = L // QB     # 8
NKT = L // P      # 8 k-tiles
NEC = E // P      # 8 feature chunks
VS = H * (DH + 1)  # 1040 v columns per k-chunk (65 per head)
LN_EPS = 1e-5

F32 = mybir.dt.float32
BF16 = mybir.dt.bfloat16
AF = mybir.ActivationFunctionType
OP = mybir.AluOpType


def _emit(nc, tc, io):
    xT, kTin, vTin = io["xT"], io["kTin"], io["vTin"]
    xnat = io["xnat"]
    wqT_d, wkT_d, wvT_d, woT_d = io["wqT"], io["wkT"], io["wvT"], io["woT"]
    bqk_d, brow_d = io["bqk"], io["brow"]
    y_out, w_out = io["y_out"], io["w_out"]

    ctx = tc.ctx
    ctx.enter_context(nc.allow_low_precision("bf16 attention"))

    const = ctx.enter_context(tc.tile_pool(name="const", bufs=1))
    persist = ctx.enter_context(tc.tile_pool(name="persist", bufs=1))

    ident_f = const.tile([P, P], F32)
    make_identity(nc, ident_f[:])
    ident = const.tile([P, P], BF16)
    nc.vector.tensor_copy(ident[:], ident_f[:])
    identH = const.tile([P, P], BF16)  # I/H for the attn-weights mean
    nc.scalar.mul(identH[:], ident_f[:], 1.0 / H)
    ones1 = const.tile([1, P], BF16)
    nc.vector.memset(ones1[:], 1.0)
    eps_sb = const.tile([P, 1], F32)
    nc.vector.memset(eps_sb[:], LN_EPS)
    # hoist the single activation-table load to t=0 (ScalarE idle)
    scratch1 = const.tile([1, 1], F32)
    nc.scalar.activation(scratch1[:], eps_sb[0:1, 0:1], AF.Exp)

    # bqk: [128, 16] f32 (cols 0-7 = bq chunk m, cols 8-15 = bk chunk m)
    bqk = const.tile([P, 2 * NEC], F32)
    # brow: [1, 4096] bf16 = bv | bo | gamma | beta
    brow = const.tile([1, 4 * E], BF16)

    qT = persist.tile([P, NEC * L], BF16)
    kT = persist.tile([P, NEC * L], BF16)
    v_sb = persist.tile([P, NKT * VS], BF16)
    woT = persist.tile([P, NEC * E], BF16)
    gamma_bc = persist.tile([P, E], BF16)
    beta_bc = persist.tile([P, E], BF16)

    # ones columns of v_sb (denominator trick): one strided memset
    nc.vector.memset(
        v_sb[:].rearrange("p (n d) -> p n d", d=DH + 1)[:, :, DH:DH + 1], 1.0)

    # ---------------- phase 1: projections ----------------
    with tc.tile_pool(name="ld", bufs=2) as ld_pool, \
         tc.tile_pool(name="wt", bufs=2) as wt_pool, \
         tc.tile_pool(name="pp", bufs=4, space="PSUM") as pp_pool:

        # order: k first, then v, then q (phase 2 needs kT/v first; q-proj
        # n=0 halves emitted before n=1 so early q-blocks can start).
        for ti, (src, w_d) in enumerate([(kTin, wkT_d), (vTin, wvT_d), (xT, wqT_d)]):
            aT = ld_pool.tile([P, NEC * L], BF16, tag="ld", name=f"aT_{ti}")
            wt = wt_pool.tile([P, NEC * E], BF16, tag="wt", name=f"wt_{ti}")
            if ti == 0:
                # quarter DMAs (chunk-halves x column-halves) ordered so the
                # first matmuls start after two 0.5MB transfers and the PE
                # never outruns the stream
                def wq(cs, hs):
                    return (wt[:].rearrange("p (c e) -> p c e", e=E)[:, cs, hs:hs + 512],
                            w_d.rearrange("(c p) e -> p c e", p=P)[:, cs, hs:hs + 512])
                def aq(cs, hs):
                    return (aT[:].rearrange("p (c l) -> p c l", l=L)[:, cs, hs:hs + 512],
                            src.rearrange("(c p) l -> p c l", p=P)[:, cs, hs:hs + 512])
                c01, c23, c03 = slice(0, 2), slice(2, 4), slice(0, 4)
                c45, c67, c47 = slice(4, 6), slice(6, 8), slice(4, 8)
                order = [wq(c01, 0), aq(c01, 0), wq(c23, 0), aq(c23, 0),
                         wq(c45, 0), aq(c45, 0), wq(c67, 0), aq(c67, 0),
                         aq(c03, 512), aq(c47, 512), wq(c03, 512), wq(c47, 512)]
                for o, i in order:
                    nc.sync.dma_start(out=o, in_=i)
                nc.sync.dma_start(out=bqk[:], in_=bqk_d[:, :])
                for r in range(4):
                    nc.sync.dma_start(out=brow[:, E * r:E * (r + 1)],
                                      in_=brow_d[r:r + 1, :])
            else:
                nc.sync.dma_start(
                    out=aT[:].rearrange("p (c l) -> p c l", l=L),
                    in_=src.rearrange("(c p) l -> p c l", p=P))
                nc.sync.dma_start(
                    out=wt[:].rearrange("p (c e) -> p c e", e=E),
                    in_=w_d.rearrange("(c p) e -> p c e", p=P))
            if ti == 0:
                # match the wtL/aTL/aTR/wtR DMA order above
                tiles16 = ([(m, 0) for m in range(4)] + [(m, 1) for m in range(4)]
                           + [(m, 0) for m in range(4, NEC)]
                           + [(m, 1) for m in range(4, NEC)])
            else:
                tiles16 = [(m, n) for n in range(2) for m in range(NEC)]
            for g in range(0, 16, 4):
                grp = tiles16[g:g + 4]
                psums = [
                    pp_pool.tile([P, 512], F32, tag="pp", name=f"pp_{ti}_{g}_{i}")
                    for i in range(len(grp))
                ]
                for c in range(NEC):
                    for i, (m, n) in enumerate(grp):
                        if ti != 1:  # q/k: feature-major out [e', l]
                            lhsT = wt[:, E * c + P * m: E * c + P * (m + 1)]
                            rhs = aT[:, L * c + 512 * n: L * c + 512 * (n + 1)]
                        else:        # v: token-major out [l, e']
                            lhsT = aT[:, L * c + P * m: L * c + P * (m + 1)]
                            rhs = wt[:, E * c + 512 * n: E * c + 512 * (n + 1)]
                        nc.tensor.matmul(
                            psums[i][:], lhsT, rhs,
                            start=(c == 0), stop=(c == NEC - 1),
                        )
                evict_order = list(enumerate(grp))
                if ti == 2 and g == 12:
                    # last group: evict the tiles whose psum banks the
                    # first phase-2 scores tile reuses first
                    evict_order = evict_order[::-1]
                for i, (m, n) in evict_order:
                    if ti != 1:
                        # bqk cols: 0-7 = bq, 8-15 = bk
                        dst_t = kT if ti == 0 else qT
                        bcol = NEC + m if ti == 0 else m
                        dst = dst_t[:, L * m + 512 * n: L * m + 512 * (n + 1)]
                        if ti == 2 and n == 1 and m % 2 == 0:
                            # split late q-proj evicts DVE/ACT so neither
                            # engine delays the first q-blocks
                            nc.vector.tensor_scalar_add(
                                out=dst, in0=psums[i][:],
                                scalar1=bqk[:, bcol:bcol + 1],
                            )
                        else:
                            nc.scalar.activation(
                                dst, psums[i][:], AF.Identity,
                                bias=bqk[:, bcol:bcol + 1],
                            )
                    else:
                        # v bias folded into the residual host-side
                        # (sum(probs)=1 makes it a constant shift of attn)
                        dst = v_sb[:, VS * m + 520 * n: VS * m + 520 * (n + 1)]
                        nc.vector.tensor_copy(
                            out=dst.rearrange("p (h d) -> p h d", d=DH + 1)[:, :, 0:DH],
                            in_=psums[i][:].rearrange("p (h d) -> p h d", d=DH),
                        )

        # out-proj weights (loaded during phase 1 tail)
        for c in range(NEC):
            nc.sync.dma_start(out=woT[:, E * c:E * (c + 1)],
                              in_=woT_d[P * c:P * (c + 1), :])
        # gamma/beta broadcast on gpsimd
        nc.gpsimd.partition_broadcast(gamma_bc[:], brow[0:1, 2 * E:3 * E])
        nc.gpsimd.partition_broadcast(beta_bc[:], brow[0:1, 3 * E:4 * E])

    # ---------------- phase 2: attention + out_proj + LN ----------------
    with tc.tile_pool(name="wnp", bufs=1, space="PSUM") as wn_pool, \
         tc.tile_pool(name="avp", bufs=1, space="PSUM") as av_pool, \
         tc.tile_pool(name="scp", bufs=3, space="PSUM") as sc_pool, \
         tc.tile_pool(name="expp", bufs=7) as exp_pool, \
         tc.tile_pool(name="prp", bufs=18) as probs_pool, \
         tc.tile_pool(name="atp", bufs=3) as attnT_pool, \
         tc.tile_pool(name="avsp", bufs=3) as avs_pool, \
         tc.tile_pool(name="ibp", bufs=12) as invbc_pool, \
         tc.tile_pool(name="ivp", bufs=3) as inv_pool, \
         tc.tile_pool(name="xqp", bufs=2) as xq_pool, \
         tc.tile_pool(name="yp", bufs=2) as y_pool, \
         tc.tile_pool(name="acq", bufs=2) as accq_pool, \
         tc.tile_pool(name="wnat", bufs=2) as wnat_pool, \
         tc.tile_pool(name="small", bufs=2) as small:

        SKEW_AV = 3    # av(h) emitted at slot h+3 (covers ScalarE backlog
        # around group ends, where the av eviction queues behind the exps)
        SKEW_W = 8     # W-acc pass A (h) at slot h+8 (after normalize)

        def emit_scores(qb, state, h):
            q0 = QB * qb
            hb, hc = (h % 2) * DH, h // 2
            sc = sc_pool.tile([P, L], F32, tag="sc", name=f"sc_{qb}_{h}")
            state["scs"].append(sc)
            for kt in range(NKT):
                nc.tensor.matmul(
                    sc[:, P * kt:P * (kt + 1)],
                    kT[hb:hb + DH, L * hc + P * kt: L * hc + P * (kt + 1)],
                    qT[hb:hb + DH, L * hc + q0: L * hc + q0 + QB],
                    start=True, stop=True,
                )
            expT = exp_pool.tile([P, L], BF16, tag="expT", name=f"expT_{qb}_{h}")
            state["exps"].append(expT)
            nc.scalar.activation(expT[:], sc[:], AF.Exp, scale=0.125)

        def emit_av(qb, state, h):
            exps, av4s, avss, inv4s, invbcs, probs = (
                state["exps"], state["av4s"], state["avss"],
                state["inv4s"], state["invbcs"], state["probs"],
            )
            g, hi = h // 4, h % 4
            if hi == 0:
                av4 = av_pool.tile([DH + 1, 4 * QB], F32, tag="av",
                                   name=f"av_{qb}_{g}")
                av4s.append(av4)
                inv4 = inv_pool.tile([1, 4 * QB], BF16, tag="inv",
                                     name=f"inv_{qb}_{g}")
                inv4s.append(inv4)
                avs = avs_pool.tile([DH, 4 * QB], BF16, tag="avs",
                                    name=f"avs_{qb}_{g}")
                avss.append(avs)
            av4, inv4, avs = av4s[g], inv4s[g], avss[g]
            expT = exps[h]
            for kt in range(NKT):
                nc.tensor.matmul(
                    av4[:, QB * hi:QB * (hi + 1)],
                    v_sb[:, VS * kt + (DH + 1) * h: VS * kt + (DH + 1) * (h + 1)],
                    expT[:, QB * kt:QB * (kt + 1)],
                    start=(kt == 0), stop=(kt == NKT - 1),
                )
            if hi == 3:
                # group complete: reciprocals, evict av to SBUF (frees the
                # single psum slot), broadcasts, normalize, attnT
                nc.vector.reciprocal(inv4[:], av4[DH:DH + 1, :])
                nc.scalar.copy(avs[:], av4[0:DH, :])
                for hh in range(4 * g, 4 * g + 4):
                    hhi = hh % 4
                    ib = invbc_pool.tile([P, QB], BF16, tag="ib",
                                         name=f"ib_{qb}_{hh}")
                    invbcs.append(ib)
                    nc.gpsimd.partition_broadcast(
                        ib[:], inv4[0:1, QB * hhi:QB * (hhi + 1)])
                for hh in range(4 * g, 4 * g + 4):
                    hhb, hhc = (hh % 2) * DH, hh // 2
                    hhi = hh % 4
                    ib = invbcs[hh]
                    pr = probs_pool.tile([P, L], BF16, tag="pr",
                                         name=f"pr_{qb}_{hh}")
                    probs.append(pr)
                    iap = ib[:]
                    bc_ap = bass.AP(
                        tensor=iap.tensor, offset=iap.offset,
                        ap=[iap.ap[0], [0, NKT], iap.ap[1]],
                    )
                    nc.vector.tensor_tensor(
                        out=pr[:].rearrange("p (n d) -> p n d", d=QB),
                        in0=exps[hh][:].rearrange("p (n d) -> p n d", d=QB),
                        in1=bc_ap, op=OP.mult,
                    )
                    nc.gpsimd.tensor_tensor(
                        out=state["attnT"][hhb:hhb + DH, QB * hhc:QB * (hhc + 1)],
                        in0=avs[:, QB * hhi:QB * (hhi + 1)],
                        in1=ib[0:DH, :], op=OP.mult,
                    )

        NH = NQB * H  # 128 global head indices; one flat pipeline, no
        # per-q-block drain: the next block's scores start while the
        # previous block's softmax/W tail is still in flight.

        states = {}

        def get_state(qb):
            if qb not in states:
                q0 = QB * qb
                x_qb = xq_pool.tile([P, E], F32, tag="xq", name=f"xq_{qb}")
                nc.sync.dma_start(out=x_qb[:], in_=xnat[q0:q0 + QB, :])
                attnT = attnT_pool.tile([P, NEC * QB], BF16, tag="attnT",
                                        name=f"attnT_{qb}")
                states[qb] = dict(scs=[], exps=[], av4s=[], avss=[],
                                  inv4s=[], invbcs=[], probs=[], attnT=attnT,
                                  x_qb=x_qb, wnA=None)
            return states[qb]

        def extras_po_y(qb_e):
            st = states.get(qb_e)
            # out_proj burst + residual add
            po = sc_pool.tile([P, L], F32, tag="sc", name=f"po_{qb_e}")
            attnT = st["attnT"]
            for eb in range(2):
                for c in range(NEC):
                    nc.tensor.matmul(
                        po[:, 512 * eb:512 * (eb + 1)],
                        attnT[:, QB * c:QB * (c + 1)],
                        woT[:, E * c + 512 * eb: E * c + 512 * (eb + 1)],
                        start=(c == 0), stop=(c == NEC - 1),
                    )
            y_sb = y_pool.tile([P, E], F32, tag="y", name=f"y_{qb_e}")
            st["y_sb"] = y_sb
            for eb in range(2):
                nc.vector.tensor_tensor(
                    out=y_sb[:, 512 * eb:512 * (eb + 1)],
                    in0=po[:, 512 * eb:512 * (eb + 1)],
                    in1=st["x_qb"][:, 512 * eb:512 * (eb + 1)],
                    op=OP.add,
                )

        def extras_w(qb_e):
            # W: pass A eviction, pass B burst + eviction
            st = states.get(qb_e)
            accq = accq_pool.tile([P, L], BF16, tag="accq",
                                  name=f"accq_{qb_e}")
            st["accq"] = accq
            nc.scalar.copy(accq[:, 0:512], st["wnA"][:])
            if qb_e == NQB - 1:
                # last block: store the finished half immediately
                nc.scalar.dma_start(
                    out=w_out[QB * qb_e:QB * (qb_e + 1), 0:512],
                    in_=accq[:, 0:512])
            wnB = wn_pool.tile([P, 512], F32, tag="wn", name=f"wnB_{qb_e}")
            for h in range(H):
                nc.tensor.matmul(
                    wnB[:], identH[:], st["probs"][h][:, 512:1024],
                    start=(h == 0), stop=(h == H - 1),
                )
            nc.scalar.copy(accq[:, 512:1024], wnB[:])

        def extras_wdma(qb_e):
            # store W in k-major block layout; host reassembles. The last
            # block's store goes through the ScalarE queue so it is not
            # head-of-line blocked behind the final y stores.
            st = states.get(qb_e)
            if qb_e == NQB - 1:
                nc.scalar.dma_start(
                    out=w_out[QB * qb_e:QB * (qb_e + 1), 512:1024],
                    in_=st["accq"][:, 512:1024])
            else:
                nc.sync.dma_start(out=w_out[QB * qb_e:QB * (qb_e + 1), :],
                                  in_=st["accq"][:])
            del states[qb_e]

        def extras_ytail(qb_e):
            st = states.get(qb_e)
            q0e = QB * qb_e
            if True:
                # y tail: stats, LN, gamma/beta halves (DVE || Pool)
                y_sb = st["y_sb"]
                stats = small.tile([P, 2, 6], F32, tag="stats",
                                   name=f"st_{qb_e}")
                yg = y_sb[:].rearrange("p (s f) -> p s f", f=512)
                for sg in range(2):
                    nc.vector.bn_stats(out=stats[:, sg, :], in_=yg[:, sg, :])
                mv = small.tile([P, 2], F32, tag="mv", name=f"mv_{qb_e}")
                nc.vector.bn_aggr(out=mv[:], in_=stats[:])
                lnv = small.tile([P, 1], F32, tag="lnv", name=f"lnv_{qb_e}")
                nc.scalar.activation(lnv[:], mv[:, 1:2], AF.Ln, bias=eps_sb[:])
                rstd = small.tile([P, 1], F32, tag="rstd", name=f"rstd_{qb_e}")
                nc.scalar.activation(rstd[:], lnv[:], AF.Exp, scale=-0.5)
                y_bf = y_pool.tile([P, E], BF16, tag="ybf", name=f"ybf_{qb_e}")
                # gamma/beta split DVE || Pool; the last block gives Pool a
                # smaller share (nothing left to hide its latency behind)
                cut = 768 if qb_e == NQB - 1 else 512
                for sl, eng in ((slice(0, cut), nc.vector),
                                (slice(cut, E), nc.gpsimd)):
                    nc.vector.tensor_scalar(
                        out=y_sb[:, sl], in0=y_sb[:, sl],
                        scalar1=mv[:, 0:1], scalar2=rstd[:],
                        op0=OP.subtract, op1=OP.mult,
                    )
                    eng.tensor_tensor(
                        out=y_sb[:, sl], in0=y_sb[:, sl],
                        in1=gamma_bc[:, sl], op=OP.mult)
                    eng.tensor_tensor(
                        out=y_bf[:, sl], in0=y_sb[:, sl],
                        in1=beta_bc[:, sl], op=OP.add)
                    nc.sync.dma_start(out=y_out[q0e:q0e + QB, sl],
                                      in_=y_bf[:, sl])

        # steady-state extras fire at js = 16*qb + 18 + ph with the PE-heavy
        # work (out_proj + W passes) at ph 6; the last block has no
        # successor to overlap with, so its y path fires as soon as its
        # attnT completes
        for js in range(NH + 2 * H):
            off = js - H - 2  # qb whose window-end extras fire at this js
            qb_e, ph = divmod(off, H)
            if 0 <= qb_e < NQB - 1:
                if ph == 6:
                    extras_po_y(qb_e)
                    extras_w(qb_e)
                elif ph == 8:
                    extras_ytail(qb_e)
                elif ph == 9:
                    extras_wdma(qb_e)
            elif qb_e == NQB - 1:
                if ph == 0:
                    extras_po_y(qb_e)
                elif ph == 1:
                    extras_ytail(qb_e)
                elif ph == 4:
                    extras_w(qb_e)
                elif ph == 5:
                    extras_wdma(qb_e)

            ja = js - SKEW_AV
            group_end = 0 <= ja < NH and ja % 4 == 3
            if group_end:
                emit_av(ja // H, get_state(ja // H), ja % H)
            if js < NH:
                emit_scores(js // H, get_state(js // H), js % H)
            if not group_end and 0 <= ja < NH:
                emit_av(ja // H, get_state(ja // H), ja % H)
            # W pass A (kt 0-3); heads 0-1 deferred to the h2 slot so the
            # first wnA write follows the previous block's passB on PE
            jw = js - SKEW_W
            if 0 <= jw < NH:
                qb_w, h_w = divmod(jw, H)
                heads = [] if h_w < 3 else ([0, 1, 2, 3] if h_w == 3 else [h_w])
                if qb_w == NQB - 1:
                    # last block: compress the W-acc tail (its pass B has
                    # nothing to overlap with)
                    if h_w == 13:
                        heads = [13, 14, 15]
                    elif h_w > 13:
                        heads = []
                st = get_state(qb_w) if heads else None
                for hw in heads:
                    if hw == 0:
                        st["wnA"] = wn_pool.tile([P, 512], F32, tag="wn",
                                                 name=f"wnA_{qb_w}")
                    nc.tensor.matmul(
                        st["wnA"][:], identH[:], st["probs"][hw][:, 0:512],
                        start=(hw == 0), stop=(hw == H - 1),
                    )


_CACHED = None


def _build():
    global _CACHED
    if _CACHED is not None:
        return _CACHED
    nc = bacc.Bacc("TRN2", target_bir_lowering=False, debug=False, num_devices=8)
    io = {}
    for name in ["xT", "kTin", "vTin", "wqT", "wkT", "wvT", "woT"]:
        io[name] = nc.dram_tensor(name, [1024, 1024], BF16, kind="ExternalInput").ap()
    io["xnat"] = nc.dram_tensor("xnat", [1024, 1024], F32, kind="ExternalInput").ap()
    io["bqk"] = nc.dram_tensor("bqk", [128, 16], F32, kind="ExternalInput").ap()
    io["brow"] = nc.dram_tensor("brow", [4, 1024], BF16, kind="ExternalInput").ap()
    io["y_out"] = nc.dram_tensor("y_out", [1024, 1024], BF16, kind="ExternalOutput").ap()
    io["w_out"] = nc.dram_tensor("w_out", [1024, 1024], BF16, kind="ExternalOutput").ap()
    with tile.TileContext(nc) as tc:
        with ExitStack() as ctx:
            tc.ctx = ctx
            _emit(nc, tc, io)
    nc.compile()
    _CACHED = nc
    return nc


def kernel(query, key_t, value, in_proj_w, in_proj_b, out_proj_w, out_proj_b,
           ln_gamma, ln_beta, _trace=False, _tmpdir=None):
    import ml_dtypes
    bf16 = ml_dtypes.bfloat16

    query = np.ascontiguousarray(np.asarray(query, dtype=np.float32))
    key_t = np.asarray(key_t, dtype=np.float32)
    value = np.asarray(value, dtype=np.float32)
    # residual carries the out_proj bias AND the value bias's constant
    # contribution (sum(probs) = 1 -> attn = probs@v_core + bv, and
    # (bv @ wo^T) is a constant row): y = (query + bo + wo@bv) + attn@woT
    out_proj_w = np.asarray(out_proj_w, dtype=np.float32)
    bv_host = np.asarray(in_proj_b, np.float32)[2 * E:3 * E]
    xres = np.ascontiguousarray(
        query + (np.asarray(out_proj_b, np.float32)
                 + out_proj_w @ bv_host)[None, None, :])
    xT = np.ascontiguousarray(np.swapaxes(query, 1, 2)).astype(bf16)
    kT = np.ascontiguousarray(np.swapaxes(key_t, 1, 2)).astype(bf16)
    vT = np.ascontiguousarray(np.swapaxes(value, 1, 2)).astype(bf16)

    in_proj_w = np.asarray(in_proj_w, dtype=np.float32)
    wqT = np.ascontiguousarray(in_proj_w[0:E].T).astype(bf16)
    wkT = np.ascontiguousarray(in_proj_w[E:2 * E].T).astype(bf16)
    wvT = np.ascontiguousarray(in_proj_w[2 * E:3 * E].T).astype(bf16)
    woT = np.ascontiguousarray(np.asarray(out_proj_w, dtype=np.float32).T).astype(bf16)

    b = np.asarray(in_proj_b, dtype=np.float32)
    bq, bk, bv = b[0:E], b[E:2 * E], b[2 * E:3 * E]
    bqk = np.ascontiguousarray(
        np.concatenate([bq.reshape(NEC, P).T, bk.reshape(NEC, P).T], axis=1)
    ).astype(np.float32)  # [128, 16]
    brow = np.ascontiguousarray(np.stack([
        bv, np.asarray(out_proj_b, np.float32),
        np.asarray(ln_gamma, np.float32), np.asarray(ln_beta, np.float32),
    ])).astype(bf16)  # [4, 1024]

    nc = _build()
    in_maps = [
        dict(xT=xT[c], kTin=kT[c], vTin=vT[c], xnat=xres[c],
             wqT=wqT, wkT=wkT, wvT=wvT, woT=woT, bqk=bqk, brow=brow)
        for c in range(8)
    ]
    res = run_bass_kernel_spmd(
        nc, in_maps, core_ids=list(range(8)), trace=_trace, tmpdir=_tmpdir
    )
    y = np.stack([r["y_out"] for r in res.results]).astype(np.float32)
    # w_out rows hold k-major blocks: w_raw[qb*128+p, kt*128+qq] =
    # W[qb*128+qq, kt*128+p]
    w_raw = np.stack([r["w_out"] for r in res.results]).astype(np.float32)
    w = np.ascontiguousarray(
        w_raw.reshape(8, NQB, P, NKT, P).transpose(0, 1, 4, 3, 2)
        .reshape(8, L, L))
    kernel._last_result = res
    return y, w
